# revision 8
# baseline (speedup 1.0000x reference)
"""Locally-connected graph-conv kernel for Trainium2 (Bass/Tile).

Computes out[b,t,m] = sum_n x[b,t,n] * (S*W)[n,m] + bias[m] for
x [64, 2048, 208], W/S [208, 208], bias [208].

The ring-graph support S is a +-4 band (mod 208), so each half of the
output nodes only needs a 112-row slice of the contraction dim:
  block 0 (m 0..103):   n in {204..207} ++ {0..107}
  block 1 (m 104..207): n in {100..207} ++ {0..3}
Each output block is a [112,104] x [112,512] matmul with the masked
weight block stationary and x^T streaming as the moving operand.

Memory-bound, so everything streams in bf16 (host casts x and the
pre-masked S*W; PSUM accumulates fp32; the eviction converts back to
bf16), halving HBM traffic both ways and staying well inside the 2e-2
rel-err envelope.

Pipeline structure (tuned against perfetto traces; steady state sits
at the ~420 B/ns aggregate DMA ceiling, so the shaping below keeps
every phase near that ceiling):
 - Column chunks are TAPERED [512,512,1024, 2048 x6, 1024,512,512]:
   light first chunks start the PE ~3us earlier, light last chunks
   shrink the serial drain after the final load.
 - Loads ride the Sync HWDGE ring; the first three chunks' block-1
   loads ride the then-idle Scalar ring so the front fills at
   two-queue rate. Load emission is hoisted AHEAD chunks past compute.
 - Matmuls per PSUM group run paired (ps0 x2 then ps1 x2) so each
   eviction unblocks as early as possible and the PE switches
   stationary weights half as often.
 - PSUM->SBUF evictions split per block: block0 on VectorE
   (tensor_scalar add of bias), block1 on the Activation engine
   (Identity+bias). Stores issue at PSUM-group granularity right
   after each eviction: block1 on the Scalar ring (its own engine's
   product), block0 on the GpSimd SWDGE queue, except the last two
   chunks' block-0 stores which ride the by-then-idle Sync ring.

Data-parallel over 8 NeuronCores: each core gets 16384 rows of the
flattened x, host-pre-assembled into a [224, 16384] bf16 tensor (two
112-row halo blocks; partition counts stay multiples of 16 for the
fast HWDGE path; stores carry 8 pad rows per block, dropped at host
gather). The host transposes y^T back at gather.
"""

import numpy as np
import ml_dtypes
from contextlib import ExitStack

import concourse.bacc as bacc
import concourse.mybir as mybir
import concourse.tile as tile
from concourse.bass_utils import run_bass_kernel_spmd

N = 208                      # nodes
HALF = 104                   # output nodes per block
K = 4                        # band half-width of S
NH = 2 * K + HALF            # 112 contraction rows per block (halo incl.)
NP = 112                     # padded store rows (multiple of 16)
N_CORES = 8
B, T = 64, 2048
ROWS_TOTAL = B * T           # 131072
SHARD = ROWS_TOTAL // N_CORES    # 16384 rows per core
TB = 512                     # moving-block columns per matmul (fp32 PSUM max)
GMAX = 1024                  # eviction/store group (2 PSUM banks)
AHEAD = 3                    # chunks of load prefetch hoisted past compute
FRONT2Q = 5                  # leading chunks whose block-1 load rides Scalar
TAILSYNC = 3                 # trailing chunks whose block-0 store rides Sync

# tapered chunk schedule (columns per chunk), sums to SHARD
CHUNKS = [512, 512, 1024] + [2048] * 6 + [1024, 512, 256, 256]
assert sum(CHUNKS) == SHARD

FP32 = mybir.dt.float32
BF16 = mybir.dt.bfloat16
NP_BF16 = np.dtype(ml_dtypes.bfloat16)

# halo row order (indices into the [208] node dim) for each block
ROWS0 = list(range(N - K, N)) + list(range(0, HALF + K))          # 112
ROWS1 = list(range(HALF - K, N)) + list(range(0, K))              # 112

_CACHE = {}
LAST_RESULTS = None          # BassKernelResults of the most recent run


def _kernel_body(tc):
    nc = tc.nc
    # rows 0:112 block0 halo, 112:224 block1 halo
    x_d = nc.dram_tensor("xh", [2 * NH, SHARD], BF16, kind="ExternalInput").ap()
    w_d = nc.dram_tensor("wh", [NH, N], BF16, kind="ExternalInput").ap()
    b_d = nc.dram_tensor("bias", [HALF, 2], FP32, kind="ExternalInput").ap()
    o_d = nc.dram_tensor("outt", [2 * NP, SHARD], BF16, kind="ExternalOutput").ap()

    starts = [sum(CHUNKS[:i]) for i in range(len(CHUNKS))]
    NCH = len(CHUNKS)

    with ExitStack() as ctx:
        const = ctx.enter_context(tc.tile_pool(name="const", bufs=1))

        # One-time setup on the Scalar HWDGE ring (idle at startup):
        # host-pre-masked stationary weight blocks wh0/wh1 [112, 104] in
        # halo row order, bias columns [104, 1] per block.
        wh = const.tile([NH, N], BF16, tag="wh")
        nc.scalar.dma_start(wh, w_d)
        bcols = const.tile([HALF, 2], FP32, tag="bcols")
        nc.scalar.dma_start(bcols, b_d)
        wh0 = wh[:, 0:HALF]
        wh1 = wh[:, HALF:N]
        bA = bcols[:, 0:1]
        bB = bcols[:, 1:2]

        x0p = ctx.enter_context(tc.tile_pool(name="x0p", bufs=6))
        x1p = ctx.enter_context(tc.tile_pool(name="x1p", bufs=6))
        o0p = ctx.enter_context(tc.tile_pool(name="o0p", bufs=4))
        o1p = ctx.enter_context(tc.tile_pool(name="o1p", bufs=4))
        ps0p = ctx.enter_context(tc.tile_pool(name="ps0p", bufs=2, space="PSUM"))
        ps1p = ctx.enter_context(tc.tile_pool(name="ps1p", bufs=2, space="PSUM"))

        xtiles = {}

        def emit_load(c):
            clen = CHUNKS[c]
            tsl = slice(starts[c], starts[c] + clen)
            xh0 = x0p.tile([NH, 2048], BF16, tag="xh0")
            xh1 = x1p.tile([NH, 2048], BF16, tag="xh1")
            nc.sync.dma_start(xh0[:, 0:clen], x_d[0:NH, tsl])
            if c < FRONT2Q:
                nc.scalar.dma_start(xh1[:, 0:clen], x_d[NH : 2 * NH, tsl])
            else:
                nc.sync.dma_start(xh1[:, 0:clen], x_d[NH : 2 * NH, tsl])
            xtiles[c] = (xh0, xh1)

        for c in range(min(AHEAD, NCH)):
            emit_load(c)

        for c in range(NCH):
            clen = CHUNKS[c]
            cstart = starts[c]
            xh0, xh1 = xtiles.pop(c)
            for g0 in range(0, clen, GMAX):
                glen = min(GMAX, clen - g0)
                gsl = slice(cstart + g0, cstart + g0 + glen)
                ps0 = ps0p.tile([HALF, GMAX], FP32, tag="ps0")
                ps1 = ps1p.tile([HALF, GMAX], FP32, tag="ps1")
                for m0 in range(0, glen, TB):
                    mlen = min(TB, glen - m0)
                    msl = slice(g0 + m0, g0 + m0 + mlen)
                    psl = slice(m0, m0 + mlen)
                    nc.tensor.matmul(
                        ps0[:, psl], wh0, xh0[:, msl], start=True, stop=True
                    )
                for m0 in range(0, glen, TB):
                    mlen = min(TB, glen - m0)
                    msl = slice(g0 + m0, g0 + m0 + mlen)
                    psl = slice(m0, m0 + mlen)
                    nc.tensor.matmul(
                        ps1[:, psl], wh1, xh1[:, msl], start=True, stop=True
                    )
                # eviction + per-partition bias, split across engines;
                # each block's store issues right after its eviction
                o0_t = o0p.tile([NP, GMAX], BF16, tag="o0")
                o1_t = o1p.tile([NP, GMAX], BF16, tag="o1")
                nc.vector.tensor_scalar_add(
                    o0_t[0:HALF, 0:glen], ps0[:, 0:glen], bA
                )
                nc.scalar.add(o1_t[0:HALF, 0:glen], ps1[:, 0:glen], bB)
                if c >= NCH - TAILSYNC:
                    nc.sync.dma_start(o_d[0:NP, gsl], o0_t[:, 0:glen])
                else:
                    nc.gpsimd.dma_start(o_d[0:NP, gsl], o0_t[:, 0:glen])
                nc.scalar.dma_start(o_d[NP : 2 * NP, gsl], o1_t[:, 0:glen])
            if c + AHEAD < NCH:
                emit_load(c + AHEAD)


def _build():
    nc = bacc.Bacc(
        "TRN2",
        target_bir_lowering=False,
        debug=False,
        num_devices=N_CORES,
    )
    with tile.TileContext(nc) as tc:
        _kernel_body(tc)
    nc.compile()
    return nc


def kernel(x, W, b, S):
    global LAST_RESULTS
    nc = _CACHE.get("nc")
    if nc is None:
        nc = _build()
        _CACHE["nc"] = nc

    xf = np.asarray(x, np.float32).reshape(ROWS_TOTAL, N)
    Mf = np.asarray(S, np.float32) * np.asarray(W, np.float32)
    Mh = Mf.astype(NP_BF16)
    wh = np.empty((NH, N), NP_BF16)
    wh[:, 0:HALF] = Mh[ROWS0][:, 0:HALF]
    wh[:, HALF:N] = Mh[ROWS1][:, HALF:N]
    bf = np.asarray(b, np.float32)
    bcols = np.empty((HALF, 2), np.float32)
    bcols[:, 0] = bf[0:HALF]
    bcols[:, 1] = bf[HALF:N]

    xb = xf.astype(NP_BF16)
    in_maps = []
    for i in range(N_CORES):
        xt = xb[i * SHARD : (i + 1) * SHARD].T          # [208, SHARD] view
        xh = np.empty((2 * NH, SHARD), NP_BF16)
        xh[0:NH] = xt[ROWS0]
        xh[NH : 2 * NH] = xt[ROWS1]
        in_maps.append({"xh": xh, "wh": wh, "bias": bcols})
    res = run_bass_kernel_spmd(nc, in_maps, core_ids=list(range(N_CORES)))
    LAST_RESULTS = res
    out = np.empty((ROWS_TOTAL, N), np.float32)
    for i, r in enumerate(res.results):
        yt = r["outt"]                                  # [224, SHARD] bf16
        out[i * SHARD : (i + 1) * SHARD, 0:HALF] = yt[0:HALF].T.astype(np.float32)
        out[i * SHARD : (i + 1) * SHARD, HALF:N] = yt[NP : NP + HALF].T.astype(
            np.float32
        )
    return out.reshape(B, T, N)


# revision 10
# speedup vs baseline: 1.0010x; 1.0010x over previous
"""Locally-connected graph-conv kernel for Trainium2 (Bass/Tile).

Computes out[b,t,m] = sum_n x[b,t,n] * (S*W)[n,m] + bias[m] for
x [64, 2048, 208], W/S [208, 208], bias [208].

The ring-graph support S is a +-4 band (mod 208), so each half of the
output nodes only needs a 112-row slice of the contraction dim:
  block 0 (m 0..103):   n in {204..207} ++ {0..107}
  block 1 (m 104..207): n in {100..207} ++ {0..3}
Each output block is a [112,104] x [112,512] matmul with the masked
weight block stationary and x^T streaming as the moving operand.

Memory-bound, so everything streams in bf16 (host casts x and the
pre-masked S*W; PSUM accumulates fp32; the eviction converts back to
bf16), halving HBM traffic both ways and staying well inside the 2e-2
rel-err envelope.

Pipeline structure (tuned against perfetto traces; steady state sits
at the ~420 B/ns aggregate DMA ceiling, so the shaping below keeps
every phase near that ceiling):
 - Column chunks are TAPERED [512,512,1024, 2048 x6, 1024,512,512]:
   light first chunks start the PE ~3us earlier, light last chunks
   shrink the serial drain after the final load.
 - Loads ride the Sync HWDGE ring; the first three chunks' block-1
   loads ride the then-idle Scalar ring so the front fills at
   two-queue rate. Load emission is hoisted AHEAD chunks past compute.
 - Matmuls per PSUM group run paired (ps0 x2 then ps1 x2) so each
   eviction unblocks as early as possible and the PE switches
   stationary weights half as often.
 - PSUM->SBUF evictions split per block: block0 on VectorE
   (tensor_scalar add of bias), block1 on the Activation engine
   (Identity+bias). Stores issue at PSUM-group granularity right
   after each eviction: block1 on the Scalar ring (its own engine's
   product), block0 on the GpSimd SWDGE queue, except the last two
   chunks' block-0 stores which ride the by-then-idle Sync ring.

Data-parallel over 8 NeuronCores: each core gets 16384 rows of the
flattened x, host-pre-assembled into a [224, 16384] bf16 tensor (two
112-row halo blocks; partition counts stay multiples of 16 for the
fast HWDGE path on loads; stores are 104 partitions, no pad rows).
The host transposes y^T back at gather.
"""

import numpy as np
import ml_dtypes
from contextlib import ExitStack

import concourse.bacc as bacc
import concourse.mybir as mybir
import concourse.tile as tile
from concourse.bass_utils import run_bass_kernel_spmd

N = 208                      # nodes
HALF = 104                   # output nodes per block
K = 4                        # band half-width of S
NH = 2 * K + HALF            # 112 contraction rows per block (halo incl.)
NP = 112                     # padded store rows (multiple of 16)
N_CORES = 8
B, T = 64, 2048
ROWS_TOTAL = B * T           # 131072
SHARD = ROWS_TOTAL // N_CORES    # 16384 rows per core
TB = 512                     # moving-block columns per matmul (fp32 PSUM max)
GMAX = 1024                  # eviction/store group (2 PSUM banks)
AHEAD = 3                    # chunks of load prefetch hoisted past compute
FRONT2Q = 3                  # leading chunks whose block-1 load rides Scalar
TAILSYNC = 2                 # trailing chunks whose block-0 store rides Sync

# tapered chunk schedule (columns per chunk), sums to SHARD
CHUNKS = [512, 512, 1024] + [2048] * 6 + [1024, 512, 512]
assert sum(CHUNKS) == SHARD

FP32 = mybir.dt.float32
BF16 = mybir.dt.bfloat16
NP_BF16 = np.dtype(ml_dtypes.bfloat16)

# halo row order (indices into the [208] node dim) for each block
ROWS0 = list(range(N - K, N)) + list(range(0, HALF + K))          # 112
ROWS1 = list(range(HALF - K, N)) + list(range(0, K))              # 112

_CACHE = {}
LAST_RESULTS = None          # BassKernelResults of the most recent run


def _kernel_body(tc):
    nc = tc.nc
    # rows 0:112 block0 halo, 112:224 block1 halo
    x_d = nc.dram_tensor("xh", [2 * NH, SHARD], BF16, kind="ExternalInput").ap()
    w_d = nc.dram_tensor("wh", [NH, N], BF16, kind="ExternalInput").ap()
    b_d = nc.dram_tensor("bias", [HALF, 2], FP32, kind="ExternalInput").ap()
    o_d = nc.dram_tensor("outt", [N, SHARD], BF16, kind="ExternalOutput").ap()

    starts = [sum(CHUNKS[:i]) for i in range(len(CHUNKS))]
    NCH = len(CHUNKS)

    with ExitStack() as ctx:
        const = ctx.enter_context(tc.tile_pool(name="const", bufs=1))

        # One-time setup on the Scalar HWDGE ring (idle at startup):
        # host-pre-masked stationary weight blocks wh0/wh1 [112, 104] in
        # halo row order, bias columns [104, 1] per block.
        wh = const.tile([NH, N], BF16, tag="wh")
        nc.scalar.dma_start(wh, w_d)
        bcols = const.tile([HALF, 2], FP32, tag="bcols")
        nc.scalar.dma_start(bcols, b_d)
        wh0 = wh[:, 0:HALF]
        wh1 = wh[:, HALF:N]
        bA = bcols[:, 0:1]
        bB = bcols[:, 1:2]

        x0p = ctx.enter_context(tc.tile_pool(name="x0p", bufs=6))
        x1p = ctx.enter_context(tc.tile_pool(name="x1p", bufs=6))
        o0p = ctx.enter_context(tc.tile_pool(name="o0p", bufs=4))
        o1p = ctx.enter_context(tc.tile_pool(name="o1p", bufs=4))
        ps0p = ctx.enter_context(tc.tile_pool(name="ps0p", bufs=2, space="PSUM"))
        ps1p = ctx.enter_context(tc.tile_pool(name="ps1p", bufs=2, space="PSUM"))

        xtiles = {}

        def emit_load(c):
            clen = CHUNKS[c]
            tsl = slice(starts[c], starts[c] + clen)
            xh0 = x0p.tile([NH, 2048], BF16, tag="xh0")
            xh1 = x1p.tile([NH, 2048], BF16, tag="xh1")
            nc.sync.dma_start(xh0[:, 0:clen], x_d[0:NH, tsl])
            if c < FRONT2Q:
                nc.scalar.dma_start(xh1[:, 0:clen], x_d[NH : 2 * NH, tsl])
            else:
                nc.sync.dma_start(xh1[:, 0:clen], x_d[NH : 2 * NH, tsl])
            xtiles[c] = (xh0, xh1)

        for c in range(min(AHEAD, NCH)):
            emit_load(c)

        for c in range(NCH):
            clen = CHUNKS[c]
            cstart = starts[c]
            xh0, xh1 = xtiles.pop(c)
            for g0 in range(0, clen, GMAX):
                glen = min(GMAX, clen - g0)
                gsl = slice(cstart + g0, cstart + g0 + glen)
                ps0 = ps0p.tile([HALF, GMAX], FP32, tag="ps0")
                ps1 = ps1p.tile([HALF, GMAX], FP32, tag="ps1")
                for m0 in range(0, glen, TB):
                    mlen = min(TB, glen - m0)
                    msl = slice(g0 + m0, g0 + m0 + mlen)
                    psl = slice(m0, m0 + mlen)
                    nc.tensor.matmul(
                        ps0[:, psl], wh0, xh0[:, msl], start=True, stop=True
                    )
                for m0 in range(0, glen, TB):
                    mlen = min(TB, glen - m0)
                    msl = slice(g0 + m0, g0 + m0 + mlen)
                    psl = slice(m0, m0 + mlen)
                    nc.tensor.matmul(
                        ps1[:, psl], wh1, xh1[:, msl], start=True, stop=True
                    )
                # eviction + per-partition bias, split across engines;
                # each block's store issues right after its eviction
                o0_t = o0p.tile([HALF, GMAX], BF16, tag="o0")
                o1_t = o1p.tile([HALF, GMAX], BF16, tag="o1")
                nc.vector.tensor_scalar_add(o0_t[:, 0:glen], ps0[:, 0:glen], bA)
                nc.scalar.add(o1_t[:, 0:glen], ps1[:, 0:glen], bB)
                if c >= NCH - TAILSYNC:
                    nc.sync.dma_start(o_d[0:HALF, gsl], o0_t[:, 0:glen])
                else:
                    nc.gpsimd.dma_start(o_d[0:HALF, gsl], o0_t[:, 0:glen])
                nc.scalar.dma_start(o_d[HALF:N, gsl], o1_t[:, 0:glen])
            if c + AHEAD < NCH:
                emit_load(c + AHEAD)


def _build():
    nc = bacc.Bacc(
        "TRN2",
        target_bir_lowering=False,
        debug=False,
        num_devices=N_CORES,
    )
    with tile.TileContext(nc) as tc:
        _kernel_body(tc)
    nc.compile()
    return nc


def kernel(x, W, b, S):
    global LAST_RESULTS
    nc = _CACHE.get("nc")
    if nc is None:
        nc = _build()
        _CACHE["nc"] = nc

    xf = np.asarray(x, np.float32).reshape(ROWS_TOTAL, N)
    Mf = np.asarray(S, np.float32) * np.asarray(W, np.float32)
    Mh = Mf.astype(NP_BF16)
    wh = np.empty((NH, N), NP_BF16)
    wh[:, 0:HALF] = Mh[ROWS0][:, 0:HALF]
    wh[:, HALF:N] = Mh[ROWS1][:, HALF:N]
    bf = np.asarray(b, np.float32)
    bcols = np.empty((HALF, 2), np.float32)
    bcols[:, 0] = bf[0:HALF]
    bcols[:, 1] = bf[HALF:N]

    xb = xf.astype(NP_BF16)
    in_maps = []
    for i in range(N_CORES):
        xt = xb[i * SHARD : (i + 1) * SHARD].T          # [208, SHARD] view
        xh = np.empty((2 * NH, SHARD), NP_BF16)
        xh[0:NH] = xt[ROWS0]
        xh[NH : 2 * NH] = xt[ROWS1]
        in_maps.append({"xh": xh, "wh": wh, "bias": bcols})
    res = run_bass_kernel_spmd(nc, in_maps, core_ids=list(range(N_CORES)))
    LAST_RESULTS = res
    out = np.empty((ROWS_TOTAL, N), np.float32)
    for i, r in enumerate(res.results):
        yt = r["outt"]                                  # [208, SHARD] bf16
        out[i * SHARD : (i + 1) * SHARD] = yt.T.astype(np.float32)
    return out.reshape(B, T, N)


# revision 11
# speedup vs baseline: 1.0175x; 1.0164x over previous
"""Locally-connected graph-conv kernel for Trainium2 (Bass/Tile).

Computes out[b,t,m] = sum_n x[b,t,n] * (S*W)[n,m] + bias[m] for
x [64, 2048, 208], W/S [208, 208], bias [208].

The ring-graph support S is a +-4 band (mod 208), so each half of the
output nodes only needs a 112-row slice of the contraction dim:
  block 0 (m 0..103):   n in {204..207} ++ {0..107}
  block 1 (m 104..207): n in {100..207} ++ {0..3}
Each output block is a [112,104] x [112,512] matmul with the masked
weight block stationary and x^T streaming as the moving operand.

Memory-bound, so everything streams in bf16 (host casts x and the
pre-masked S*W; PSUM accumulates fp32; the eviction converts back to
bf16), halving HBM traffic both ways and staying well inside the 2e-2
rel-err envelope.

Pipeline structure (tuned against perfetto traces; steady state sits
at the ~420 B/ns aggregate DMA ceiling, so the shaping below keeps
every phase near that ceiling):
 - Column chunks are TAPERED [512,512,1024, 2048 x6, 1024,512,512]:
   light first chunks start the PE ~3us earlier, light last chunks
   shrink the serial drain after the final load.
 - Loads ride the Sync HWDGE ring; the first three chunks' block-1
   loads ride the then-idle Scalar ring so the front fills at
   two-queue rate. Load emission is hoisted AHEAD chunks past compute.
 - Matmuls per PSUM group run paired (ps0 x2 then ps1 x2) so each
   eviction unblocks as early as possible and the PE switches
   stationary weights half as often.
 - PSUM->SBUF evictions split per block: block0 on VectorE
   (tensor_scalar add of bias), block1 on the Activation engine
   (Identity+bias). Stores issue at PSUM-group granularity right
   after each eviction: block1 on the Scalar ring (its own engine's
   product), block0 on the GpSimd SWDGE queue, except the last two
   chunks' block-0 stores which ride the by-then-idle Sync ring.

Data-parallel over 8 NeuronCores: each core gets 16384 rows of the
flattened x, host-pre-assembled into a [224, 16384] bf16 tensor (two
112-row halo blocks; partition counts stay multiples of 16 for the
fast HWDGE path; stores carry 8 pad rows per block, dropped at host
gather). The host transposes y^T back at gather.
"""

import numpy as np
import ml_dtypes
from contextlib import ExitStack

import concourse.bacc as bacc
import concourse.mybir as mybir
import concourse.tile as tile
from concourse.bass_utils import run_bass_kernel_spmd

N = 208                      # nodes
HALF = 104                   # output nodes per block
K = 4                        # band half-width of S
NH = 2 * K + HALF            # 112 contraction rows per block (halo incl.)
NP = 112                     # padded store rows (multiple of 16)
N_CORES = 8
B, T = 64, 2048
ROWS_TOTAL = B * T           # 131072
SHARD = ROWS_TOTAL // N_CORES    # 16384 rows per core
TB = 512                     # moving-block columns per matmul (fp32 PSUM max)
GMAX = 1024                  # eviction/store group (2 PSUM banks)
AHEAD = 3                    # chunks of load prefetch hoisted past compute
FRONT2Q = 3                  # leading chunks whose block-1 load rides Scalar
TAILSYNC = 2                 # trailing chunks whose block-0 store rides Sync

# tapered chunk schedule (columns per chunk), sums to SHARD
CHUNKS = [512, 512, 1024] + [2048] * 6 + [1024, 512, 512]
assert sum(CHUNKS) == SHARD

FP32 = mybir.dt.float32
BF16 = mybir.dt.bfloat16
NP_BF16 = np.dtype(ml_dtypes.bfloat16)

# halo row order (indices into the [208] node dim) for each block
ROWS0 = list(range(N - K, N)) + list(range(0, HALF + K))          # 112
ROWS1 = list(range(HALF - K, N)) + list(range(0, K))              # 112

_CACHE = {}
LAST_RESULTS = None          # BassKernelResults of the most recent run


def _kernel_body(tc):
    nc = tc.nc
    # rows 0:112 block0 halo, 112:224 block1 halo
    x_d = nc.dram_tensor("xh", [2 * NH, SHARD], BF16, kind="ExternalInput").ap()
    w_d = nc.dram_tensor("wh", [NH, N], BF16, kind="ExternalInput").ap()
    b_d = nc.dram_tensor("bias", [HALF, 2], FP32, kind="ExternalInput").ap()
    o_d = nc.dram_tensor("outt", [2 * NP, SHARD], BF16, kind="ExternalOutput").ap()

    starts = [sum(CHUNKS[:i]) for i in range(len(CHUNKS))]
    NCH = len(CHUNKS)

    with ExitStack() as ctx:
        const = ctx.enter_context(tc.tile_pool(name="const", bufs=1))

        # One-time setup on the Scalar HWDGE ring (idle at startup):
        # host-pre-masked stationary weight blocks wh0/wh1 [112, 104] in
        # halo row order, bias columns [104, 1] per block.
        wh = const.tile([NH, N], BF16, tag="wh")
        nc.scalar.dma_start(wh, w_d)
        bcols = const.tile([HALF, 2], FP32, tag="bcols")
        nc.scalar.dma_start(bcols, b_d)
        wh0 = wh[:, 0:HALF]
        wh1 = wh[:, HALF:N]
        bA = bcols[:, 0:1]
        bB = bcols[:, 1:2]

        x0p = ctx.enter_context(tc.tile_pool(name="x0p", bufs=6))
        x1p = ctx.enter_context(tc.tile_pool(name="x1p", bufs=6))
        o0p = ctx.enter_context(tc.tile_pool(name="o0p", bufs=4))
        o1p = ctx.enter_context(tc.tile_pool(name="o1p", bufs=4))
        ps0p = ctx.enter_context(tc.tile_pool(name="ps0p", bufs=2, space="PSUM"))
        ps1p = ctx.enter_context(tc.tile_pool(name="ps1p", bufs=2, space="PSUM"))

        xtiles = {}

        def emit_load(c):
            clen = CHUNKS[c]
            tsl = slice(starts[c], starts[c] + clen)
            xh0 = x0p.tile([NH, 2048], BF16, tag="xh0")
            xh1 = x1p.tile([NH, 2048], BF16, tag="xh1")
            nc.sync.dma_start(xh0[:, 0:clen], x_d[0:NH, tsl])
            if c < FRONT2Q:
                nc.scalar.dma_start(xh1[:, 0:clen], x_d[NH : 2 * NH, tsl])
            else:
                nc.sync.dma_start(xh1[:, 0:clen], x_d[NH : 2 * NH, tsl])
            xtiles[c] = (xh0, xh1)

        for c in range(min(AHEAD, NCH)):
            emit_load(c)

        for c in range(NCH):
            clen = CHUNKS[c]
            cstart = starts[c]
            xh0, xh1 = xtiles.pop(c)
            for g0 in range(0, clen, GMAX):
                glen = min(GMAX, clen - g0)
                gsl = slice(cstart + g0, cstart + g0 + glen)
                ps0 = ps0p.tile([HALF, GMAX], FP32, tag="ps0")
                ps1 = ps1p.tile([HALF, GMAX], FP32, tag="ps1")
                for m0 in range(0, glen, TB):
                    mlen = min(TB, glen - m0)
                    msl = slice(g0 + m0, g0 + m0 + mlen)
                    psl = slice(m0, m0 + mlen)
                    nc.tensor.matmul(
                        ps0[:, psl], wh0, xh0[:, msl], start=True, stop=True
                    )
                for m0 in range(0, glen, TB):
                    mlen = min(TB, glen - m0)
                    msl = slice(g0 + m0, g0 + m0 + mlen)
                    psl = slice(m0, m0 + mlen)
                    nc.tensor.matmul(
                        ps1[:, psl], wh1, xh1[:, msl], start=True, stop=True
                    )
                # eviction + per-partition bias, split across engines;
                # each block's store issues right after its eviction
                o0_t = o0p.tile([NP, GMAX], BF16, tag="o0")
                o1_t = o1p.tile([NP, GMAX], BF16, tag="o1")
                nc.vector.tensor_scalar_add(
                    o0_t[0:HALF, 0:glen], ps0[:, 0:glen], bA
                )
                nc.scalar.add(o1_t[0:HALF, 0:glen], ps1[:, 0:glen], bB)
                if c >= NCH - TAILSYNC:
                    nc.sync.dma_start(o_d[0:NP, gsl], o0_t[:, 0:glen])
                else:
                    nc.gpsimd.dma_start(o_d[0:NP, gsl], o0_t[:, 0:glen])
                nc.scalar.dma_start(o_d[NP : 2 * NP, gsl], o1_t[:, 0:glen])
            if c + AHEAD < NCH:
                emit_load(c + AHEAD)


def _build():
    nc = bacc.Bacc(
        "TRN2",
        target_bir_lowering=False,
        debug=False,
        num_devices=N_CORES,
    )
    with tile.TileContext(nc) as tc:
        _kernel_body(tc)
    nc.compile()
    return nc


def kernel(x, W, b, S):
    global LAST_RESULTS
    nc = _CACHE.get("nc")
    if nc is None:
        nc = _build()
        _CACHE["nc"] = nc

    xf = np.asarray(x, np.float32).reshape(ROWS_TOTAL, N)
    Mf = np.asarray(S, np.float32) * np.asarray(W, np.float32)
    Mh = Mf.astype(NP_BF16)
    wh = np.empty((NH, N), NP_BF16)
    wh[:, 0:HALF] = Mh[ROWS0][:, 0:HALF]
    wh[:, HALF:N] = Mh[ROWS1][:, HALF:N]
    bf = np.asarray(b, np.float32)
    bcols = np.empty((HALF, 2), np.float32)
    bcols[:, 0] = bf[0:HALF]
    bcols[:, 1] = bf[HALF:N]

    xb = xf.astype(NP_BF16)
    in_maps = []
    for i in range(N_CORES):
        xt = xb[i * SHARD : (i + 1) * SHARD].T          # [208, SHARD] view
        xh = np.empty((2 * NH, SHARD), NP_BF16)
        xh[0:NH] = xt[ROWS0]
        xh[NH : 2 * NH] = xt[ROWS1]
        in_maps.append({"xh": xh, "wh": wh, "bias": bcols})
    res = run_bass_kernel_spmd(nc, in_maps, core_ids=list(range(N_CORES)))
    LAST_RESULTS = res
    out = np.empty((ROWS_TOTAL, N), np.float32)
    for i, r in enumerate(res.results):
        yt = r["outt"]                                  # [224, SHARD] bf16
        out[i * SHARD : (i + 1) * SHARD, 0:HALF] = yt[0:HALF].T.astype(np.float32)
        out[i * SHARD : (i + 1) * SHARD, HALF:N] = yt[NP : NP + HALF].T.astype(
            np.float32
        )
    return out.reshape(B, T, N)


# revision 13
# speedup vs baseline: 1.0400x; 1.0221x over previous
"""Locally-connected graph-conv kernel for Trainium2 (Bass/Tile).

Computes out[b,t,m] = sum_n x[b,t,n] * (S*W)[n,m] + bias[m] for
x [64, 2048, 208], W/S [208, 208], bias [208].

The ring-graph support S is a +-4 band (mod 208), so each half of the
output nodes only needs a 112-row slice of the contraction dim:
  block 0 (m 0..103):   n in {204..207} ++ {0..107}
  block 1 (m 104..207): n in {100..207} ++ {0..3}
Each output block is a [112,104] x [112,512] matmul with the masked
weight block stationary and x^T streaming as the moving operand.

Memory-bound, so everything streams in bf16 (host casts x and the
pre-masked S*W; PSUM accumulates fp32; the eviction converts back to
bf16), halving HBM traffic both ways and staying well inside the 2e-2
rel-err envelope.

Pipeline structure (tuned against perfetto traces; steady state sits
at the ~420 B/ns aggregate DMA ceiling, so the shaping below keeps
every phase near that ceiling):
 - Column chunks are TAPERED [512,512,1024, 2048 x6, 1024,512,512]:
   light first chunks start the PE ~3us earlier, light last chunks
   shrink the serial drain after the final load.
 - Loads ride the Sync HWDGE ring; the first three chunks' block-1
   loads ride the then-idle Scalar ring so the front fills at
   two-queue rate. Load emission is hoisted AHEAD chunks past compute.
 - Matmuls per PSUM group run paired (ps0 x2 then ps1 x2) so each
   eviction unblocks as early as possible and the PE switches
   stationary weights half as often.
 - PSUM->SBUF evictions split per block: block0 on VectorE
   (tensor_scalar add of bias), block1 on the Activation engine
   (Identity+bias). Stores issue at PSUM-group granularity right
   after each eviction: block1 on the Scalar ring (its own engine's
   product), block0 on the GpSimd SWDGE queue, except the last two
   chunks' block-0 stores which ride the by-then-idle Sync ring.

Data-parallel over 8 NeuronCores: each core gets 16384 rows of the
flattened x, host-pre-assembled into a [224, 16384] bf16 tensor (two
112-row halo blocks; partition counts stay multiples of 16 for the
fast HWDGE path; stores carry 8 pad rows per block, dropped at host
gather). The host transposes y^T back at gather.
"""

import numpy as np
import ml_dtypes
from contextlib import ExitStack

import concourse.bacc as bacc
import concourse.mybir as mybir
import concourse.tile as tile
from concourse.bass_utils import run_bass_kernel_spmd

N = 208                      # nodes
HALF = 104                   # output nodes per block
K = 4                        # band half-width of S
NH = 2 * K + HALF            # 112 contraction rows per block (halo incl.)
NP = 112                     # padded store rows (multiple of 16)
N_CORES = 8
B, T = 64, 2048
ROWS_TOTAL = B * T           # 131072
SHARD = ROWS_TOTAL // N_CORES    # 16384 rows per core
TB = 512                     # moving-block columns per matmul (fp32 PSUM max)
GMAX = 1024                  # eviction/store group (2 PSUM banks)
AHEAD = 3                    # chunks of load prefetch hoisted past compute
FRONT2Q = 3                  # leading chunks whose block-1 load rides Scalar
TAILSYNC = 2                 # trailing chunks whose block-0 store rides Sync

# tapered chunk schedule (columns per chunk), sums to SHARD
CHUNKS = [512, 512, 1024] + [2048] * 6 + [1024, 512, 512]
assert sum(CHUNKS) == SHARD

FP32 = mybir.dt.float32
BF16 = mybir.dt.bfloat16
NP_BF16 = np.dtype(ml_dtypes.bfloat16)

# halo row order (indices into the [208] node dim) for each block
ROWS0 = list(range(N - K, N)) + list(range(0, HALF + K))          # 112
ROWS1 = list(range(HALF - K, N)) + list(range(0, K))              # 112

_CACHE = {}
LAST_RESULTS = None          # BassKernelResults of the most recent run


def _kernel_body(tc):
    nc = tc.nc
    # rows 0:112 block0 halo, 112:224 block1 halo
    x_d = nc.dram_tensor("xh", [2 * NH, SHARD], BF16, kind="ExternalInput").ap()
    w_d = nc.dram_tensor("wh", [NH, N], BF16, kind="ExternalInput").ap()
    b_d = nc.dram_tensor("bias", [HALF, 2], FP32, kind="ExternalInput").ap()
    o_d = nc.dram_tensor("outt", [2 * NP, SHARD], BF16, kind="ExternalOutput").ap()

    starts = [sum(CHUNKS[:i]) for i in range(len(CHUNKS))]
    NCH = len(CHUNKS)

    with ExitStack() as ctx:
        const = ctx.enter_context(tc.tile_pool(name="const", bufs=1))

        # One-time setup on the Scalar HWDGE ring (idle at startup):
        # host-pre-masked stationary weight blocks wh0/wh1 [112, 104] in
        # halo row order, bias columns [104, 1] per block.
        wh = const.tile([NH, N], BF16, tag="wh")
        nc.scalar.dma_start(wh, w_d)
        # bias rides the GpSimd queue so the Scalar ring's front-chunk
        # load configs aren't delayed behind its slow 8B-line config
        bcols = const.tile([HALF, 2], FP32, tag="bcols")
        nc.gpsimd.dma_start(bcols, b_d)
        wh0 = wh[:, 0:HALF]
        wh1 = wh[:, HALF:N]
        bA = bcols[:, 0:1]
        bB = bcols[:, 1:2]

        x0p = ctx.enter_context(tc.tile_pool(name="x0p", bufs=6))
        x1p = ctx.enter_context(tc.tile_pool(name="x1p", bufs=6))
        o0p = ctx.enter_context(tc.tile_pool(name="o0p", bufs=4))
        o1p = ctx.enter_context(tc.tile_pool(name="o1p", bufs=4))
        ps0p = ctx.enter_context(tc.tile_pool(name="ps0p", bufs=2, space="PSUM"))
        ps1p = ctx.enter_context(tc.tile_pool(name="ps1p", bufs=2, space="PSUM"))

        # PE p-state warm-up: the Tensor engine only reaches full clock
        # after ~3us of continuous execution, and the first real matmul
        # can't start until the first x chunk lands (~9.5us). Run a burst
        # of throwaway matmuls on memset scratch during that dead window,
        # timed so the last one ends as real data arrives — the busy
        # stretch then continues seamlessly into real matmuls at full
        # clock instead of mid p-state.
        wsx = const.tile([NH, TB], BF16, tag="wsx")
        wsw = const.tile([NH, HALF], BF16, tag="wsw")
        nc.vector.memset(wsx, 0.0)
        nc.vector.memset(wsw, 0.0)
        for _ in range(7):
            psw = ps0p.tile([HALF, GMAX], FP32, tag="ps0")
            nc.tensor.matmul(psw[:, 0:TB], wsw, wsx, start=True, stop=True)

        xtiles = {}

        def emit_load(c):
            clen = CHUNKS[c]
            tsl = slice(starts[c], starts[c] + clen)
            xh0 = x0p.tile([NH, 2048], BF16, tag="xh0")
            xh1 = x1p.tile([NH, 2048], BF16, tag="xh1")
            nc.sync.dma_start(xh0[:, 0:clen], x_d[0:NH, tsl])
            if c < FRONT2Q:
                nc.scalar.dma_start(xh1[:, 0:clen], x_d[NH : 2 * NH, tsl])
            else:
                nc.sync.dma_start(xh1[:, 0:clen], x_d[NH : 2 * NH, tsl])
            xtiles[c] = (xh0, xh1)

        for c in range(min(AHEAD, NCH)):
            emit_load(c)

        for c in range(NCH):
            clen = CHUNKS[c]
            cstart = starts[c]
            xh0, xh1 = xtiles.pop(c)
            for g0 in range(0, clen, GMAX):
                glen = min(GMAX, clen - g0)
                gsl = slice(cstart + g0, cstart + g0 + glen)
                ps0 = ps0p.tile([HALF, GMAX], FP32, tag="ps0")
                ps1 = ps1p.tile([HALF, GMAX], FP32, tag="ps1")
                for m0 in range(0, glen, TB):
                    mlen = min(TB, glen - m0)
                    msl = slice(g0 + m0, g0 + m0 + mlen)
                    psl = slice(m0, m0 + mlen)
                    nc.tensor.matmul(
                        ps0[:, psl], wh0, xh0[:, msl], start=True, stop=True
                    )
                for m0 in range(0, glen, TB):
                    mlen = min(TB, glen - m0)
                    msl = slice(g0 + m0, g0 + m0 + mlen)
                    psl = slice(m0, m0 + mlen)
                    nc.tensor.matmul(
                        ps1[:, psl], wh1, xh1[:, msl], start=True, stop=True
                    )
                # eviction + per-partition bias, split across engines;
                # each block's store issues right after its eviction
                o0_t = o0p.tile([NP, GMAX], BF16, tag="o0")
                o1_t = o1p.tile([NP, GMAX], BF16, tag="o1")
                nc.vector.tensor_scalar_add(
                    o0_t[0:HALF, 0:glen], ps0[:, 0:glen], bA
                )
                nc.scalar.add(o1_t[0:HALF, 0:glen], ps1[:, 0:glen], bB)
                if c >= NCH - TAILSYNC:
                    nc.sync.dma_start(o_d[0:NP, gsl], o0_t[:, 0:glen])
                else:
                    nc.gpsimd.dma_start(o_d[0:NP, gsl], o0_t[:, 0:glen])
                nc.scalar.dma_start(o_d[NP : 2 * NP, gsl], o1_t[:, 0:glen])
            if c + AHEAD < NCH:
                emit_load(c + AHEAD)


def _build():
    nc = bacc.Bacc(
        "TRN2",
        target_bir_lowering=False,
        debug=False,
        num_devices=N_CORES,
    )
    with tile.TileContext(nc) as tc:
        _kernel_body(tc)
    nc.compile()
    return nc


def kernel(x, W, b, S):
    global LAST_RESULTS
    nc = _CACHE.get("nc")
    if nc is None:
        nc = _build()
        _CACHE["nc"] = nc

    xf = np.asarray(x, np.float32).reshape(ROWS_TOTAL, N)
    Mf = np.asarray(S, np.float32) * np.asarray(W, np.float32)
    Mh = Mf.astype(NP_BF16)
    wh = np.empty((NH, N), NP_BF16)
    wh[:, 0:HALF] = Mh[ROWS0][:, 0:HALF]
    wh[:, HALF:N] = Mh[ROWS1][:, HALF:N]
    bf = np.asarray(b, np.float32)
    bcols = np.empty((HALF, 2), np.float32)
    bcols[:, 0] = bf[0:HALF]
    bcols[:, 1] = bf[HALF:N]

    xb = xf.astype(NP_BF16)
    in_maps = []
    for i in range(N_CORES):
        xt = xb[i * SHARD : (i + 1) * SHARD].T          # [208, SHARD] view
        xh = np.empty((2 * NH, SHARD), NP_BF16)
        xh[0:NH] = xt[ROWS0]
        xh[NH : 2 * NH] = xt[ROWS1]
        in_maps.append({"xh": xh, "wh": wh, "bias": bcols})
    res = run_bass_kernel_spmd(nc, in_maps, core_ids=list(range(N_CORES)))
    LAST_RESULTS = res
    out = np.empty((ROWS_TOTAL, N), np.float32)
    for i, r in enumerate(res.results):
        yt = r["outt"]                                  # [224, SHARD] bf16
        out[i * SHARD : (i + 1) * SHARD, 0:HALF] = yt[0:HALF].T.astype(np.float32)
        out[i * SHARD : (i + 1) * SHARD, HALF:N] = yt[NP : NP + HALF].T.astype(
            np.float32
        )
    return out.reshape(B, T, N)
